# revision 10
# baseline (speedup 1.0000x reference)
"""ALiBi transformer layer on 8 TRN2 NeuronCores.

Sharding: tensor-parallel attention (16 heads -> 2 per core) +
sequence-parallel FFN (4096 tokens -> 512 per core, full w1/w2 replicated).
Collectives: a tiny prewarm AllGather (absorbs the NEFF-entry rank-skew
barrier), a 2-way-split bf16 AllGather of ln1(x)^T (per-rank 0.5MB each,
lets kqv start after the first half lands), and one bf16 AllToAll of
per-head attention outputs.

Matmul compute in bf16 (fp32 accumulation in PSUM); layernorms, softmax
normalization and residuals in fp32. Softmax uses a fixed max-shift
(exp(s/32 - 4)) which is safe because |scores| <= ~1 here, and the
ALiBi+causal term is folded in as a host-precomputed multiplicative
table exp(slope*(j-i)) (0 where masked), so no max-reduction and no
transposes of the attention matrix are needed. The softmax denominator
comes free from an appended ones-column in the V operand.
"""

import sys

sys.path.insert(0, "/opt/trn_rl_repo")

import math

import ml_dtypes
import numpy as np

import concourse.bass as bass
import concourse.tile as tile
from concourse import bacc, mybir
from concourse.bass_utils import run_bass_kernel_spmd
from concourse.masks import make_identity

F32 = mybir.dt.float32
BF16 = mybir.dt.bfloat16
AF = mybir.ActivationFunctionType

B, S, E = 2, 2048, 1024
H, D = 16, 64            # heads, head dim
DFF = 4096
W = 8                    # cores
T = B * S                # 4096 tokens
TPC = T // W             # 512 tokens per core
HPC = H // W             # 2 heads per core
EPS = 1e-5
SCALE = 1.0 / math.sqrt(E)
EXP_SHIFT = -4.0         # fixed softmax shift; scores are << 4 here
RG = [list(range(W))]

_CACHE = {}


def _ln(nc, pools, x_ap, g_b, b_b, out_bf, eps_t):
    """LayerNorm over free dim (1024) of x_ap [128, 1024] f32 -> out_bf bf16."""
    stats = pools.tile([128, 2, 6], F32, tag="ln_stats")
    xg = x_ap.rearrange("p (s f) -> p s f", s=2)
    for sg in range(2):
        nc.vector.bn_stats(out=stats[:, sg, :], in_=xg[:, sg, :])
    mv = pools.tile([128, 2], F32, tag="ln_mv")
    nc.vector.bn_aggr(out=mv[:], in_=stats[:])
    nc.scalar.activation(out=mv[:, 1:2], in_=mv[:, 1:2], func=AF.Sqrt,
                         bias=eps_t[:], scale=1.0)
    nc.vector.reciprocal(out=mv[:, 1:2], in_=mv[:, 1:2])
    xc = pools.tile([128, 1024], F32, tag="ln_xc")
    nc.vector.tensor_scalar(out=xc[:], in0=x_ap, scalar1=mv[:, 0:1],
                            scalar2=mv[:, 1:2],
                            op0=mybir.AluOpType.subtract,
                            op1=mybir.AluOpType.mult)
    nc.vector.tensor_mul(out=xc[:], in0=xc[:], in1=g_b[:])
    nc.vector.tensor_add(out=out_bf, in0=xc[:], in1=b_b[:])


def _build(debug=False):
    nc = bacc.Bacc(None, target_bir_lowering=False, num_devices=W)

    x_s = nc.declare_dram_parameter("x_s", [TPC, E], F32, isOutput=False)
    wqk = nc.declare_dram_parameter("wqk", [E, 2 * HPC * D], BF16, isOutput=False)
    wv = nc.declare_dram_parameter("wv", [E, HPC * D], BF16, isOutput=False)
    w1b = nc.declare_dram_parameter("w1b", [E, DFF], BF16, isOutput=False)
    w2b = nc.declare_dram_parameter("w2b", [DFF, E], BF16, isOutput=False)
    b1_d = nc.declare_dram_parameter("b1", [DFF], F32, isOutput=False)
    b2_d = nc.declare_dram_parameter("b2", [E], F32, isOutput=False)
    g1_d = nc.declare_dram_parameter("g1", [E], F32, isOutput=False)
    be1_d = nc.declare_dram_parameter("be1", [E], F32, isOutput=False)
    g2_d = nc.declare_dram_parameter("g2", [E], F32, isOutput=False)
    be2_d = nc.declare_dram_parameter("be2", [E], F32, isOutput=False)
    expb_d = nc.declare_dram_parameter("expb", [HPC, 128, 4096], BF16, isOutput=False)
    out_ext = nc.declare_dram_parameter("out", [TPC, E], F32, isOutput=True)
    if debug:
        dbg_attn = nc.declare_dram_parameter("dbg_attn", [128, 8, 4, 128], BF16,
                                             isOutput=True)

    def bcast_row(dram_ap):
        return bass.AP(tensor=dram_ap.tensor, offset=0, ap=[[0, 128], [1, E]])

    with tile.TileContext(nc) as tc:
        with (
            tc.tile_pool(name="dram", bufs=1, space="DRAM") as dram,
            tc.tile_pool(name="params", bufs=1) as prm,
            tc.tile_pool(name="persist", bufs=1) as per,
            tc.tile_pool(name="lntmp", bufs=2) as lnp,
        ):
            # ---- DRAM bounce buffers for collectives ----
            ag_in_a = dram.tile([E // 2, TPC], BF16)
            ag_in_b = dram.tile([E // 2, TPC], BF16)
            ag_out_a = dram.tile([W * E // 2, TPC], BF16, addr_space="Shared")
            ag_out_b = dram.tile([W * E // 2, TPC], BF16, addr_space="Shared")
            a2a_in = dram.tile([T, HPC * D], BF16)
            a2a_out = dram.tile([T, HPC * D], BF16)

            # ---- critical-path params first ----
            g1_b = prm.tile([128, E], F32)
            be1_b = prm.tile([128, E], F32)
            nc.sync.dma_start(out=g1_b[:], in_=bcast_row(g1_d[:]))
            nc.sync.dma_start(out=be1_b[:], in_=bcast_row(be1_d[:]))
            eps_t = prm.tile([128, 1], F32)
            nc.vector.memset(eps_t[:], EPS)
            ident_bf = prm.tile([128, 128], BF16)
            make_identity(nc, ident_bf[:])

            # ---- persistent activations ----
            x_sb = per.tile([128, 4, E], F32)        # x, then x+attn in place
            xb2_sb = per.tile([128, 4, E], F32)      # x+attn+b2 (fc2 epilogue)
            qT_sb = per.tile([128, 8, 512], BF16)    # [qk-col(2h*64), chunk, tok]
            kT_sb = per.tile([128, 8, 512], BF16)
            v_sb = per.tile([128, 8, 4, 2 * (D + 1)], BF16)  # v + ones cols
            h2T_sb = per.tile([128, 8, 512], BF16)
            aT_sb = per.tile([128, 32, 512], BF16)
            attn_sb = per.tile([128, 8, 4, 128], BF16)

            nc.vector.memset(v_sb[:], 1.0)

            # =========== Phase 1: LN1 + transpose + split AllGather ===========
            with (
                tc.tile_pool(name="h1stage", bufs=1) as h1s,
                tc.tile_pool(name="pT", bufs=4, space="PSUM") as pT,
            ):
                h1T_sb = h1s.tile([128, 8, 512], BF16)
                for tt in range(4):
                    for q4 in range(4):
                        nc.sync.dma_start(
                            out=x_sb[:, tt, 256 * q4:256 * (q4 + 1)],
                            in_=x_s[128 * tt:128 * (tt + 1), 256 * q4:256 * (q4 + 1)])
                    h1_bf = lnp.tile([128, E], BF16, tag="h1bf")
                    _ln(nc, lnp, x_sb[:, tt, :], g1_b, be1_b, h1_bf[:], eps_t)
                    for eb in range(8):
                        pt = pT.tile([128, 128], BF16)
                        nc.tensor.transpose(pt[:], h1_bf[:, 128 * eb:128 * (eb + 1)],
                                            ident_bf[:])
                        if eb % 2:
                            nc.scalar.copy(out=h1T_sb[:, eb, 128 * tt:128 * (tt + 1)],
                                           in_=pt[:])
                        else:
                            nc.vector.tensor_copy(
                                out=h1T_sb[:, eb, 128 * tt:128 * (tt + 1)], in_=pt[:])
                nc.sync.dma_start(
                    out=ag_in_a[:].rearrange("(eb p) t -> p eb t", p=128),
                    in_=h1T_sb[:, 0:4, :])
                nc.gpsimd.collective_compute(
                    "AllGather", mybir.AluOpType.bypass, replica_groups=RG,
                    ins=[ag_in_a.opt()], outs=[ag_out_a.opt()])
                nc.sync.dma_start(
                    out=ag_in_b[:].rearrange("(eb p) t -> p eb t", p=128),
                    in_=h1T_sb[:, 4:8, :])
                nc.gpsimd.collective_compute(
                    "AllGather", mybir.AluOpType.bypass, replica_groups=RG,
                    ins=[ag_in_b.opt()], outs=[ag_out_b.opt()])

            # ---- remaining params (off the phase-1 critical path) ----
            g2_b = prm.tile([128, E], F32)
            be2_b = prm.tile([128, E], F32)
            b2_b = prm.tile([128, E], F32)
            for t_, d_ in ((g2_b, g2_d), (be2_b, be2_d), (b2_b, b2_d)):
                nc.sync.dma_start(out=t_[:], in_=bcast_row(d_[:]))
            b1_sb = prm.tile([128, DFF // 128], F32)
            nc.sync.dma_start(out=b1_sb[:], in_=b1_d[:].rearrange("(j p) -> p j", p=128))
            neg4 = prm.tile([128, 1], F32)
            nc.vector.memset(neg4[:], EXP_SHIFT)
            ident_f = prm.tile([128, 128], F32)
            make_identity(nc, ident_f[:])
            expb_sb = prm.tile([128, HPC, 4096], BF16)
            nc.sync.dma_start(out=expb_sb[:],
                              in_=expb_d[:].rearrange("h p c -> p h c"))
            wqk_sb = prm.tile([128, 8, 2 * HPC * D], BF16)
            nc.sync.dma_start(out=wqk_sb[:],
                              in_=wqk[:].rearrange("(eb p) c -> p eb c", p=128))
            wv_sb = prm.tile([128, 8, HPC * D], BF16)
            nc.sync.dma_start(out=wv_sb[:],
                              in_=wv[:].rearrange("(eb p) c -> p eb c", p=128))

            ag_va = ag_out_a[:].rearrange("(r eb p) t -> r eb p t", eb=4, p=128)
            ag_vb = ag_out_b[:].rearrange("(r eb p) t -> r eb p t", eb=4, p=128)

            # =========== Phase 2: kqv projections ===========
            with (
                tc.tile_pool(name="h1t", bufs=2) as h1tp,
                tc.tile_pool(name="pkqv", bufs=2, space="PSUM") as pkqv,
            ):
                for c in range(8):
                    pk = pkqv.tile([128, 512], F32, tag="pk")
                    pq = pkqv.tile([128, 512], F32, tag="pq")
                    h1ts = []
                    for eb in range(8):
                        h1t = h1tp.tile([128, 512], BF16, name=f"h1t{eb}")
                        src = ag_va[c, eb] if eb < 4 else ag_vb[c, eb - 4]
                        nc.sync.dma_start(out=h1t[:], in_=src)
                        h1ts.append(h1t)
                        st, sp = eb == 0, eb == 7
                        nc.tensor.matmul(pk[:], wqk_sb[:, eb, 0:128], h1t[:],
                                         start=st, stop=sp)
                        nc.tensor.matmul(pq[:], wqk_sb[:, eb, 128:256], h1t[:],
                                         start=st, stop=sp)
                    nc.vector.tensor_copy(out=kT_sb[:, c, :], in_=pk[:])
                    nc.scalar.copy(out=qT_sb[:, c, :], in_=pq[:])
                    for tb in range(4):
                        # each tb gets its own psum bank: start=True clears the
                        # whole bank, so accumulation groups must not share one
                        pv = pkqv.tile([128, 128], F32, tag="pv")
                        for eb in range(8):
                            nc.tensor.matmul(
                                pv[:], h1ts[eb][:, 128 * tb:128 * (tb + 1)],
                                wv_sb[:, eb, :], start=(eb == 0), stop=(eb == 7))
                        for hh in range(HPC):
                            nc.vector.tensor_copy(
                                out=v_sb[:, c, tb, 65 * hh:65 * hh + 64],
                                in_=pv[:, 64 * hh:64 * hh + 64])

            # =========== Phase 3: attention (2-kb-block groups) ===========
            with (
                tc.tile_pool(name="ps_s", bufs=2, space="PSUM") as ps_s,
                tc.tile_pool(name="ps_o", bufs=2, space="PSUM") as ps_o,
                tc.tile_pool(name="ps_t", bufs=2, space="PSUM") as ps_t,
                tc.tile_pool(name="att_t", bufs=3) as att,
            ):
                units = []
                for gq in range(8):
                    b_, qc = gq // 4, gq % 4
                    nkb = 4 * (qc + 1)
                    for hh in range(HPC):
                        for g in range(nkb // 2):
                            units.append((gq, hh, g, nkb))

                pos = {}
                ps_of = {}

                def qk_pair(idx):
                    gq, hh, g, nkb = units[idx]
                    ps = ps_s.tile([128, 2, 512], F32, name="ps")
                    for j, kb in ((0, 2 * g + 1), (1, 2 * g)):
                        ck, tbk = 4 * (gq // 4) + kb // 4, kb % 4
                        nc.tensor.matmul(
                            ps[:, j, :],
                            kT_sb[64 * hh:64 * hh + 64, ck,
                                  128 * tbk:128 * (tbk + 1)],
                            qT_sb[64 * hh:64 * hh + 64, gq, :],
                            start=True, stop=True)
                    ps_of[idx] = ps

                def epilogue(gq, hh):
                    po = pos.pop((gq, hh))
                    o_sb = att.tile([65, 512], F32, tag="osb")
                    if (2 * gq + hh) % 2:
                        nc.scalar.copy(out=o_sb[:], in_=po[:])
                    else:
                        nc.vector.tensor_copy(out=o_sb[:], in_=po[:])
                    for tb in range(4):
                        pt = ps_t.tile([128, 65], F32)
                        nc.tensor.transpose(pt[:], o_sb[:, 128 * tb:128 * (tb + 1)],
                                            ident_f[:65, :65])
                        rc = att.tile([128, 1], F32, tag="rc")
                        nc.vector.reciprocal(out=rc[:], in_=pt[:, 64:65])
                        nc.vector.tensor_scalar_mul(
                            out=attn_sb[:, gq, tb, 64 * hh:64 * (hh + 1)],
                            in0=pt[:, 0:64], scalar1=rc[:])
                    if hh == HPC - 1:
                        nc.sync.dma_start(
                            out=a2a_in[:].rearrange("(g tb p) c -> g p tb c",
                                                    tb=4, p=128)[gq],
                            in_=attn_sb[:, gq, :, :])

                qk_pair(0)
                pending = None
                for idx, (gq, hh, g, nkb) in enumerate(units):
                    if idx + 1 < len(units):
                        qk_pair(idx + 1)
                    if (gq, hh) not in pos:
                        pos[(gq, hh)] = ps_o.tile([65, 512], F32, name="po")
                    po = pos[(gq, hh)]
                    ps = ps_of.pop(idx)
                    kb0, kb1 = 2 * g, 2 * g + 1
                    e_bf = att.tile([128, 2, 512], BF16, tag="ebf")
                    nc.scalar.activation(
                        out=e_bf[:].rearrange("p a b -> p (a b)"),
                        in_=ps[:].rearrange("p a b -> p (a b)"),
                        func=AF.Exp, bias=neg4[:], scale=SCALE)
                    p_bf = att.tile([128, 2, 512], BF16, tag="pbf")
                    # slot j=0 covers kb1, j=1 covers kb0: table offsets step
                    # +128 from off(kb1), matching slot order
                    qc = gq % 4
                    off_hi = 2048 - (128 * kb1 - 512 * qc)
                    base = expb_sb[:, hh, :]
                    gsl = bass.AP(tensor=base.tensor,
                                  offset=base.offset + off_hi,
                                  ap=[list(base.ap[0]), [128, 2], [1, 512]])
                    eng = nc.gpsimd if idx % 3 == 2 else nc.vector
                    eng.tensor_mul(out=p_bf[:], in0=e_bf[:], in1=gsl)
                    for j, kb in ((1, kb0), (0, kb1)):
                        ck, tbk = 4 * (gq // 4) + kb // 4, kb % 4
                        nc.tensor.matmul(
                            po[:],
                            v_sb[:, ck, tbk, 65 * hh:65 * (hh + 1)],
                            p_bf[:, j, :],
                            start=(kb == 0), stop=(kb == nkb - 1))
                    if pending is not None:
                        epilogue(*pending)
                        pending = None
                    if g == nkb // 2 - 1:
                        pending = (gq, hh)
                if pending is not None:
                    epilogue(*pending)
            if debug:
                nc.sync.dma_start(out=dbg_attn[:], in_=attn_sb[:])
            nc.gpsimd.collective_compute(
                "AllToAll", mybir.AluOpType.bypass, replica_groups=RG,
                ins=[a2a_in.opt()], outs=[a2a_out.opt()])

            a2a_v = a2a_out[:].rearrange("(r tk p) c -> tk p r c", r=W, p=128)

            # =========== Phase 4: residual + LN2 + transpose ===========
            with (
                tc.tile_pool(name="afull", bufs=2) as afp,
                tc.tile_pool(name="pT2", bufs=4, space="PSUM") as pT2,
            ):
                for tt in range(4):
                    af = afp.tile([128, W, 128], BF16)
                    nc.sync.dma_start(out=af[:], in_=a2a_v[tt])
                    nc.vector.tensor_add(out=x_sb[:, tt, :], in0=x_sb[:, tt, :],
                                         in1=af[:].rearrange("p r c -> p (r c)"))
                    nc.vector.tensor_add(out=xb2_sb[:, tt, :], in0=x_sb[:, tt, :],
                                         in1=b2_b[:])
                    h2_bf = lnp.tile([128, E], BF16, tag="h2bf")
                    _ln(nc, lnp, x_sb[:, tt, :], g2_b, be2_b, h2_bf[:], eps_t)
                    for eb in range(8):
                        pt = pT2.tile([128, 128], BF16)
                        nc.tensor.transpose(pt[:], h2_bf[:, 128 * eb:128 * (eb + 1)],
                                            ident_bf[:])
                        if eb % 2:
                            nc.scalar.copy(out=h2T_sb[:, eb, 128 * tt:128 * (tt + 1)],
                                           in_=pt[:])
                        else:
                            nc.vector.tensor_copy(
                                out=h2T_sb[:, eb, 128 * tt:128 * (tt + 1)], in_=pt[:])

            # =========== Phase 5: fc1 (a^T = relu(w1^T h2^T + b1)) ===========
            with (
                tc.tile_pool(name="w1t", bufs=6) as w1p,
                tc.tile_pool(name="ps_a", bufs=1, space="PSUM") as ps_a,
            ):
                for dffc in range(8):
                    pa = [ps_a.tile([128, 512], F32, name=f"pa{i}", tag=f"pa{i}")
                          for i in range(4)]
                    for eb in range(8):
                        w1t = w1p.tile([128, 512], BF16)
                        nc.sync.dma_start(
                            out=w1t[:],
                            in_=w1b[128 * eb:128 * (eb + 1),
                                    512 * dffc:512 * (dffc + 1)])
                        for db in range(4):
                            nc.tensor.matmul(pa[db][:],
                                             w1t[:, 128 * db:128 * (db + 1)],
                                             h2T_sb[:, eb, :],
                                             start=(eb == 0), stop=(eb == 7))
                    for db in range(4):
                        j = 4 * dffc + db
                        nc.scalar.activation(out=aT_sb[:, j, :], in_=pa[db][:],
                                             func=AF.Relu, bias=b1_sb[:, j:j + 1],
                                             scale=1.0)

            # =========== Phase 6: fc2 + residual ===========
            with (
                tc.tile_pool(name="w2t", bufs=6) as w2p,
                tc.tile_pool(name="ps_f", bufs=1, space="PSUM") as ps_f,
                tc.tile_pool(name="outp", bufs=4) as outp,
            ):
                pf = [[ps_f.tile([128, 512], F32, name=f"pf{i}_{j}", tag=f"pf{i}_{j}")
                       for j in range(2)] for i in range(4)]
                for db in range(32):
                    w2t = w2p.tile([128, E], BF16)
                    nc.sync.dma_start(out=w2t[:], in_=w2b[128 * db:128 * (db + 1), :])
                    for tt in range(4):
                        for eh in range(2):
                            nc.tensor.matmul(pf[tt][eh][:],
                                             aT_sb[:, db, 128 * tt:128 * (tt + 1)],
                                             w2t[:, 512 * eh:512 * (eh + 1)],
                                             start=(db == 0), stop=(db == 31))
                for tt in range(4):
                    for eh in range(2):
                        ot = outp.tile([128, 512], F32)
                        nc.vector.tensor_add(
                            out=ot[:], in0=pf[tt][eh][:],
                            in1=xb2_sb[:, tt, 512 * eh:512 * (eh + 1)])
                        nc.sync.dma_start(
                            out=out_ext[128 * tt:128 * (tt + 1),
                                        512 * eh:512 * (eh + 1)],
                            in_=ot[:])
    nc.compile()
    return nc


def _expbias_table():
    """[HPC, 128, 4096] per-core list: exp(slope*(k - c + 2048)), 0 where future."""
    kk = np.arange(128)[:, None]
    cc = np.arange(4096)[None, :]
    rel = (kk - cc + 2048).astype(np.float64)
    xslope = (2.0 ** 8) ** (1.0 / H)
    tables = []
    for r in range(W):
        heads = []
        for hh in range(HPC):
            h = 2 * r + hh
            slope = 1.0 / xslope ** (h + 1)
            with np.errstate(over="ignore"):
                g = np.where(rel <= 0, np.exp(slope * rel), 0.0)
            heads.append(g.astype(np.float32))
        tables.append(np.stack(heads).astype(ml_dtypes.bfloat16))
    return tables


def kernel(x, w_kqv, ln1_g, ln1_b, ln2_g, ln2_b, w1, b1, w2, b2, _debug=False):
    x = np.asarray(x, np.float32)
    x_flat = np.ascontiguousarray(x.reshape(T, E))
    wk = np.asarray(w_kqv, np.float32)
    bf = ml_dtypes.bfloat16
    w1_bf = np.ascontiguousarray(np.asarray(w1, np.float32).astype(bf))
    w2_bf = np.ascontiguousarray(np.asarray(w2, np.float32).astype(bf))

    key = "nc_dbg" if _debug else "nc"
    if key not in _CACHE:
        _CACHE[key] = _build(debug=_debug)
        _CACHE.setdefault("expb", _expbias_table())
    nc = _CACHE[key]
    expbs = _CACHE["expb"]

    in_maps = []
    for r in range(W):
        cols = slice(128 * r, 128 * (r + 1))
        wqk_r = np.concatenate([wk[:, 0:E][:, cols], wk[:, E:2 * E][:, cols]],
                               axis=1).astype(bf)
        wv_r = np.ascontiguousarray(wk[:, 2 * E:3 * E][:, cols]).astype(bf)
        in_maps.append({
            "x_s": np.ascontiguousarray(x_flat[TPC * r:TPC * (r + 1)]),
            "wqk": np.ascontiguousarray(wqk_r),
            "wv": wv_r,
            "w1b": w1_bf,
            "w2b": w2_bf,
            "b1": np.ascontiguousarray(np.asarray(b1, np.float32)),
            "b2": np.ascontiguousarray(np.asarray(b2, np.float32)),
            "g1": np.ascontiguousarray(np.asarray(ln1_g, np.float32)),
            "be1": np.ascontiguousarray(np.asarray(ln1_b, np.float32)),
            "g2": np.ascontiguousarray(np.asarray(ln2_g, np.float32)),
            "be2": np.ascontiguousarray(np.asarray(ln2_b, np.float32)),
            "expb": expbs[r],
        })

    _CACHE["last_in_maps"] = in_maps
    res = run_bass_kernel_spmd(nc, in_maps, core_ids=list(range(W)))
    out = np.concatenate([res.results[r]["out"] for r in range(W)], axis=0)
    if _debug:
        return out.reshape(B, S, E), res.results
    return out.reshape(B, S, E)


if __name__ == "__main__":
    import reference
    inputs = {k: np.asarray(v) for k, v in reference.setup_inputs().items()}
    got = kernel(**inputs)
    exp = np.asarray(reference.reference(**reference.setup_inputs()))
    err = np.abs(got - exp).max() / np.abs(exp).max()
    print("Relative error:", err)


# revision 12
# speedup vs baseline: 1.0774x; 1.0774x over previous
"""ALiBi transformer layer on 8 TRN2 NeuronCores.

Sharding: tensor-parallel attention (16 heads -> 2 per core) +
sequence-parallel FFN (4096 tokens -> 512 per core, full w1/w2 replicated).
Collectives: a tiny prewarm AllGather (absorbs the NEFF-entry rank-skew
barrier), a 2-way-split bf16 AllGather of ln1(x)^T (per-rank 0.5MB each,
lets kqv start after the first half lands), and one bf16 AllToAll of
per-head attention outputs.

Matmul compute in bf16 (fp32 accumulation in PSUM); layernorms, softmax
normalization and residuals in fp32. Softmax uses a fixed max-shift
(exp(s/32 - 4)) which is safe because |scores| <= ~1 here, and the
ALiBi+causal term is folded in as a host-precomputed multiplicative
table exp(slope*(j-i)) (0 where masked), so no max-reduction and no
transposes of the attention matrix are needed. The softmax denominator
comes free from an appended ones-column in the V operand.
"""

import sys

sys.path.insert(0, "/opt/trn_rl_repo")

import math

import ml_dtypes
import numpy as np

import concourse.bass as bass
import concourse.tile as tile
from concourse import bacc, mybir
from concourse.bass_utils import run_bass_kernel_spmd
from concourse.masks import make_identity

F32 = mybir.dt.float32
BF16 = mybir.dt.bfloat16
AF = mybir.ActivationFunctionType

B, S, E = 2, 2048, 1024
H, D = 16, 64            # heads, head dim
DFF = 4096
W = 8                    # cores
T = B * S                # 4096 tokens
TPC = T // W             # 512 tokens per core
HPC = H // W             # 2 heads per core
EPS = 1e-5
SCALE = 1.0 / math.sqrt(E)
EXP_SHIFT = -4.0         # fixed softmax shift; scores are << 4 here
RG = [list(range(W))]

_CACHE = {}


def _ln(nc, pools, x_ap, g_b, b_b, out_bf, eps_t):
    """LayerNorm over free dim (1024) of x_ap [128, 1024] f32 -> out_bf bf16."""
    stats = pools.tile([128, 2, 6], F32, tag="ln_stats")
    xg = x_ap.rearrange("p (s f) -> p s f", s=2)
    for sg in range(2):
        nc.vector.bn_stats(out=stats[:, sg, :], in_=xg[:, sg, :])
    mv = pools.tile([128, 2], F32, tag="ln_mv")
    nc.vector.bn_aggr(out=mv[:], in_=stats[:])
    nc.scalar.activation(out=mv[:, 1:2], in_=mv[:, 1:2], func=AF.Sqrt,
                         bias=eps_t[:], scale=1.0)
    nc.vector.reciprocal(out=mv[:, 1:2], in_=mv[:, 1:2])
    xc = pools.tile([128, 1024], F32, tag="ln_xc")
    nc.vector.tensor_scalar(out=xc[:], in0=x_ap, scalar1=mv[:, 0:1],
                            scalar2=mv[:, 1:2],
                            op0=mybir.AluOpType.subtract,
                            op1=mybir.AluOpType.mult)
    nc.vector.tensor_mul(out=xc[:], in0=xc[:], in1=g_b[:])
    nc.vector.tensor_add(out=out_bf, in0=xc[:], in1=b_b[:])


def _build(debug=False):
    nc = bacc.Bacc(None, target_bir_lowering=False, num_devices=W)

    x_s = nc.declare_dram_parameter("x_s", [TPC, E], F32, isOutput=False)
    wqk = nc.declare_dram_parameter("wqk", [E, 2 * HPC * D], BF16, isOutput=False)
    wv = nc.declare_dram_parameter("wv", [E, HPC * D], BF16, isOutput=False)
    w1b = nc.declare_dram_parameter("w1b", [E, DFF], BF16, isOutput=False)
    w2b = nc.declare_dram_parameter("w2b", [DFF, E], BF16, isOutput=False)
    b1_d = nc.declare_dram_parameter("b1", [DFF], F32, isOutput=False)
    b2_d = nc.declare_dram_parameter("b2", [E], F32, isOutput=False)
    g1_d = nc.declare_dram_parameter("g1", [E], F32, isOutput=False)
    be1_d = nc.declare_dram_parameter("be1", [E], F32, isOutput=False)
    g2_d = nc.declare_dram_parameter("g2", [E], F32, isOutput=False)
    be2_d = nc.declare_dram_parameter("be2", [E], F32, isOutput=False)
    expb_d = nc.declare_dram_parameter("expb", [HPC, 128, 4096], BF16, isOutput=False)
    out_ext = nc.declare_dram_parameter("out", [TPC, E], F32, isOutput=True)
    if debug:
        dbg_attn = nc.declare_dram_parameter("dbg_attn", [128, 8, 4, 128], BF16,
                                             isOutput=True)

    def bcast_row(dram_ap):
        return bass.AP(tensor=dram_ap.tensor, offset=0, ap=[[0, 128], [1, E]])

    with tile.TileContext(nc) as tc:
        with (
            tc.tile_pool(name="dram", bufs=1, space="DRAM") as dram,
            tc.tile_pool(name="params", bufs=1) as prm,
            tc.tile_pool(name="persist", bufs=1) as per,
            tc.tile_pool(name="lntmp", bufs=2) as lnp,
        ):
            # ---- DRAM bounce buffers for collectives ----
            ag_in_a = dram.tile([E // 2, TPC], BF16)
            ag_in_b = dram.tile([E // 2, TPC], BF16)
            ag_out_a = dram.tile([W * E // 2, TPC], BF16, addr_space="Shared")
            ag_out_b = dram.tile([W * E // 2, TPC], BF16, addr_space="Shared")
            a2a_in = dram.tile([T, HPC * D], BF16)
            a2a_out = dram.tile([T, HPC * D], BF16)

            # ---- critical-path params first ----
            g1_b = prm.tile([128, E], F32)
            be1_b = prm.tile([128, E], F32)
            nc.sync.dma_start(out=g1_b[:], in_=bcast_row(g1_d[:]))
            nc.sync.dma_start(out=be1_b[:], in_=bcast_row(be1_d[:]))
            eps_t = prm.tile([128, 1], F32)
            nc.vector.memset(eps_t[:], EPS)
            ident_bf = prm.tile([128, 128], BF16)
            make_identity(nc, ident_bf[:])

            # ---- persistent activations ----
            x_sb = per.tile([128, 4, E], F32)        # x, then x+attn in place
            xb2_sb = per.tile([128, 4, E], F32)      # x+attn+b2 (fc2 epilogue)
            qT_sb = per.tile([128, 8, 512], BF16)    # [qk-col(2h*64), chunk, tok]
            kT_sb = per.tile([128, 8, 512], BF16)
            v_sb = per.tile([128, 8, 4, 2 * (D + 1)], BF16)  # v + ones cols
            h2T_sb = per.tile([128, 8, 512], BF16)
            aT_sb = per.tile([128, 32, 512], BF16)
            attn_sb = per.tile([128, 8, 4, 128], BF16)

            nc.vector.memset(v_sb[:], 1.0)

            # =========== Phase 1: LN1 + transpose + split AllGather ===========
            with (
                tc.tile_pool(name="h1stage", bufs=1) as h1s,
                tc.tile_pool(name="pT", bufs=4, space="PSUM") as pT,
            ):
                h1T_sb = h1s.tile([128, 8, 512], BF16)
                for tt in range(4):
                    for q4 in range(4):
                        nc.sync.dma_start(
                            out=x_sb[:, tt, 256 * q4:256 * (q4 + 1)],
                            in_=x_s[128 * tt:128 * (tt + 1), 256 * q4:256 * (q4 + 1)])
                    h1_bf = lnp.tile([128, E], BF16, tag="h1bf")
                    _ln(nc, lnp, x_sb[:, tt, :], g1_b, be1_b, h1_bf[:], eps_t)
                    for eb in range(8):
                        pt = pT.tile([128, 128], BF16)
                        nc.tensor.transpose(pt[:], h1_bf[:, 128 * eb:128 * (eb + 1)],
                                            ident_bf[:])
                        if eb % 2:
                            nc.scalar.copy(out=h1T_sb[:, eb, 128 * tt:128 * (tt + 1)],
                                           in_=pt[:])
                        else:
                            nc.vector.tensor_copy(
                                out=h1T_sb[:, eb, 128 * tt:128 * (tt + 1)], in_=pt[:])
                nc.sync.dma_start(
                    out=ag_in_a[:].rearrange("(eb p) t -> p eb t", p=128),
                    in_=h1T_sb[:, 0:4, :])
                nc.gpsimd.collective_compute(
                    "AllGather", mybir.AluOpType.bypass, replica_groups=RG,
                    ins=[ag_in_a.opt()], outs=[ag_out_a.opt()])
                nc.sync.dma_start(
                    out=ag_in_b[:].rearrange("(eb p) t -> p eb t", p=128),
                    in_=h1T_sb[:, 4:8, :])
                nc.gpsimd.collective_compute(
                    "AllGather", mybir.AluOpType.bypass, replica_groups=RG,
                    ins=[ag_in_b.opt()], outs=[ag_out_b.opt()])

            # ---- remaining params (off the phase-1 critical path) ----
            g2_b = prm.tile([128, E], F32)
            be2_b = prm.tile([128, E], F32)
            b2_b = prm.tile([128, E], F32)
            for t_, d_ in ((g2_b, g2_d), (be2_b, be2_d), (b2_b, b2_d)):
                nc.sync.dma_start(out=t_[:], in_=bcast_row(d_[:]))
            b1_sb = prm.tile([128, DFF // 128], F32)
            nc.sync.dma_start(out=b1_sb[:], in_=b1_d[:].rearrange("(j p) -> p j", p=128))
            neg4 = prm.tile([128, 1], F32)
            nc.vector.memset(neg4[:], EXP_SHIFT)
            ident_f = prm.tile([128, 128], F32)
            make_identity(nc, ident_f[:])
            expb_sb = prm.tile([128, HPC, 4096], BF16)
            nc.sync.dma_start(out=expb_sb[:],
                              in_=expb_d[:].rearrange("h p c -> p h c"))
            wqk_sb = prm.tile([128, 8, 2 * HPC * D], BF16)
            nc.sync.dma_start(out=wqk_sb[:],
                              in_=wqk[:].rearrange("(eb p) c -> p eb c", p=128))
            wv_sb = prm.tile([128, 8, HPC * D], BF16)
            nc.sync.dma_start(out=wv_sb[:],
                              in_=wv[:].rearrange("(eb p) c -> p eb c", p=128))

            ag_va = ag_out_a[:].rearrange("(r eb p) t -> r eb p t", eb=4, p=128)
            ag_vb = ag_out_b[:].rearrange("(r eb p) t -> r eb p t", eb=4, p=128)

            # ====== Phases 2+3: kqv interleaved with attention ======
            # kqv chunks are PE-dense, attention is ACT/DVE-dense; interleaving
            # keeps the PE busy enough that HAM holds the 2.4 GHz clock.
            with (
                tc.tile_pool(name="h1t", bufs=2) as h1tp,
                tc.tile_pool(name="pqa", bufs=1, space="PSUM") as pqa,
                tc.tile_pool(name="att_t", bufs=4) as att,
            ):
                def kqv_chunk(c):
                    pk = pqa.tile([128, 512], F32, name="pk", tag="pk", bufs=1)
                    pq = pqa.tile([128, 512], F32, name="pq", tag="pq", bufs=1)
                    h1ts = []
                    for eb in range(8):
                        h1t = h1tp.tile([128, 512], BF16, name=f"h1t{eb}")
                        src_ = ag_va[c, eb] if eb < 4 else ag_vb[c, eb - 4]
                        nc.sync.dma_start(out=h1t[:], in_=src_)
                        h1ts.append(h1t)
                        st, sp = eb == 0, eb == 7
                        nc.tensor.matmul(pk[:], wqk_sb[:, eb, 0:128], h1t[:],
                                         start=st, stop=sp)
                        nc.tensor.matmul(pq[:], wqk_sb[:, eb, 128:256], h1t[:],
                                         start=st, stop=sp)
                    nc.vector.tensor_copy(out=kT_sb[:, c, :], in_=pk[:])
                    nc.scalar.copy(out=qT_sb[:, c, :], in_=pq[:])
                    for tb in range(4):
                        # own psum bank per tb: start=True clears a whole bank
                        pv = pqa.tile([128, 128], F32, name="pv", tag="pv", bufs=1)
                        for eb in range(8):
                            nc.tensor.matmul(
                                pv[:], h1ts[eb][:, 128 * tb:128 * (tb + 1)],
                                wv_sb[:, eb, :], start=(eb == 0), stop=(eb == 7))
                        for hh in range(HPC):
                            eng = nc.vector if (tb + hh) % 2 else nc.scalar
                            if (tb + hh) % 2:
                                nc.vector.tensor_copy(
                                    out=v_sb[:, c, tb, 65 * hh:65 * hh + 64],
                                    in_=pv[:, 64 * hh:64 * hh + 64])
                            else:
                                nc.scalar.copy(
                                    out=v_sb[:, c, tb, 65 * hh:65 * hh + 64],
                                    in_=pv[:, 64 * hh:64 * hh + 64])

                def attn_group(gq):
                    b_, qc = gq // 4, gq % 4
                    nkb = 4 * (qc + 1)
                    for hh in range(HPC):
                        po = pqa.tile([65, 512], F32, name="po", tag="po", bufs=1)
                        for kb in range(nkb):
                            ck, tbk = 4 * b_ + kb // 4, kb % 4
                            ps = pqa.tile([128, 512], F32, name="ps", tag="ps",
                                          bufs=3)
                            nc.tensor.matmul(
                                ps[:],
                                kT_sb[64 * hh:64 * hh + 64, ck,
                                      128 * tbk:128 * (tbk + 1)],
                                qT_sb[64 * hh:64 * hh + 64, gq, :],
                                start=True, stop=True)
                            e_bf = att.tile([128, 512], BF16, tag="ebf")
                            nc.scalar.activation(out=e_bf[:], in_=ps[:],
                                                 func=AF.Exp, bias=neg4[:],
                                                 scale=SCALE)
                            p_bf = att.tile([128, 512], BF16, tag="pbf")
                            off = 2048 - (128 * kb - 512 * qc)
                            nc.vector.tensor_mul(
                                out=p_bf[:], in0=e_bf[:],
                                in1=expb_sb[:, hh, off:off + 512])
                            nc.tensor.matmul(
                                po[:], v_sb[:, ck, tbk, 65 * hh:65 * (hh + 1)],
                                p_bf[:], start=(kb == 0), stop=(kb == nkb - 1))
                        o_sb = att.tile([65, 512], F32, tag="osb")
                        if (2 * gq + hh) % 2:
                            nc.scalar.copy(out=o_sb[:], in_=po[:])
                        else:
                            nc.vector.tensor_copy(out=o_sb[:], in_=po[:])
                        for tb in range(4):
                            pt = pqa.tile([128, 65], F32, name="pt", tag="pt",
                                          bufs=1)
                            nc.tensor.transpose(pt[:],
                                                o_sb[:, 128 * tb:128 * (tb + 1)],
                                                ident_f[:65, :65])
                            rc = att.tile([128, 1], F32, tag="rc")
                            nc.vector.reciprocal(out=rc[:], in_=pt[:, 64:65])
                            dst = attn_sb[:, gq, tb, 64 * hh:64 * (hh + 1)]
                            if tb % 2:
                                nc.scalar.activation(out=dst, in_=pt[:, 0:64],
                                                     func=AF.Copy, scale=rc[:])
                            else:
                                nc.vector.tensor_scalar_mul(out=dst,
                                                            in0=pt[:, 0:64],
                                                            scalar1=rc[:])
                    nc.sync.dma_start(
                        out=a2a_in[:].rearrange("(g tb p) c -> g p tb c",
                                                tb=4, p=128)[gq],
                        in_=attn_sb[:, gq, :, :])

                kqv_chunk(0)
                kqv_chunk(1)
                for gq in range(8):
                    attn_group(gq)
                    if gq + 2 < 8:
                        kqv_chunk(gq + 2)
            if debug:
                nc.sync.dma_start(out=dbg_attn[:], in_=attn_sb[:])
            nc.gpsimd.collective_compute(
                "AllToAll", mybir.AluOpType.bypass, replica_groups=RG,
                ins=[a2a_in.opt()], outs=[a2a_out.opt()])

            a2a_v = a2a_out[:].rearrange("(r tk p) c -> tk p r c", r=W, p=128)

            # =========== Phase 4: residual + LN2 + transpose ===========
            with (
                tc.tile_pool(name="afull", bufs=2) as afp,
                tc.tile_pool(name="pT2", bufs=4, space="PSUM") as pT2,
            ):
                for tt in range(4):
                    af = afp.tile([128, W, 128], BF16)
                    nc.sync.dma_start(out=af[:], in_=a2a_v[tt])
                    nc.vector.tensor_add(out=x_sb[:, tt, :], in0=x_sb[:, tt, :],
                                         in1=af[:].rearrange("p r c -> p (r c)"))
                    nc.vector.tensor_add(out=xb2_sb[:, tt, :], in0=x_sb[:, tt, :],
                                         in1=b2_b[:])
                    h2_bf = lnp.tile([128, E], BF16, tag="h2bf")
                    _ln(nc, lnp, x_sb[:, tt, :], g2_b, be2_b, h2_bf[:], eps_t)
                    for eb in range(8):
                        pt = pT2.tile([128, 128], BF16)
                        nc.tensor.transpose(pt[:], h2_bf[:, 128 * eb:128 * (eb + 1)],
                                            ident_bf[:])
                        if eb % 2:
                            nc.scalar.copy(out=h2T_sb[:, eb, 128 * tt:128 * (tt + 1)],
                                           in_=pt[:])
                        else:
                            nc.vector.tensor_copy(
                                out=h2T_sb[:, eb, 128 * tt:128 * (tt + 1)], in_=pt[:])

            # =========== Phase 5: fc1 (a^T = relu(w1^T h2^T + b1)) ===========
            with (
                tc.tile_pool(name="w1t", bufs=6) as w1p,
                tc.tile_pool(name="ps_a", bufs=1, space="PSUM") as ps_a,
            ):
                for dffc in range(8):
                    pa = [ps_a.tile([128, 512], F32, name=f"pa{i}", tag=f"pa{i}")
                          for i in range(4)]
                    for eb in range(8):
                        w1t = w1p.tile([128, 512], BF16)
                        nc.sync.dma_start(
                            out=w1t[:],
                            in_=w1b[128 * eb:128 * (eb + 1),
                                    512 * dffc:512 * (dffc + 1)])
                        for db in range(4):
                            nc.tensor.matmul(pa[db][:],
                                             w1t[:, 128 * db:128 * (db + 1)],
                                             h2T_sb[:, eb, :],
                                             start=(eb == 0), stop=(eb == 7))
                    for db in range(4):
                        j = 4 * dffc + db
                        nc.scalar.activation(out=aT_sb[:, j, :], in_=pa[db][:],
                                             func=AF.Relu, bias=b1_sb[:, j:j + 1],
                                             scale=1.0)

            # =========== Phase 6: fc2 + residual ===========
            with (
                tc.tile_pool(name="w2t", bufs=6) as w2p,
                tc.tile_pool(name="ps_f", bufs=1, space="PSUM") as ps_f,
                tc.tile_pool(name="outp", bufs=4) as outp,
            ):
                pf = [[ps_f.tile([128, 512], F32, name=f"pf{i}_{j}", tag=f"pf{i}_{j}")
                       for j in range(2)] for i in range(4)]
                for db in range(32):
                    w2t = w2p.tile([128, E], BF16)
                    nc.sync.dma_start(out=w2t[:], in_=w2b[128 * db:128 * (db + 1), :])
                    for tt in range(4):
                        for eh in range(2):
                            nc.tensor.matmul(pf[tt][eh][:],
                                             aT_sb[:, db, 128 * tt:128 * (tt + 1)],
                                             w2t[:, 512 * eh:512 * (eh + 1)],
                                             start=(db == 0), stop=(db == 31))
                for tt in range(4):
                    for eh in range(2):
                        ot = outp.tile([128, 512], F32)
                        nc.vector.tensor_add(
                            out=ot[:], in0=pf[tt][eh][:],
                            in1=xb2_sb[:, tt, 512 * eh:512 * (eh + 1)])
                        nc.sync.dma_start(
                            out=out_ext[128 * tt:128 * (tt + 1),
                                        512 * eh:512 * (eh + 1)],
                            in_=ot[:])
    nc.compile()
    return nc


def _expbias_table():
    """[HPC, 128, 4096] per-core list: exp(slope*(k - c + 2048)), 0 where future."""
    kk = np.arange(128)[:, None]
    cc = np.arange(4096)[None, :]
    rel = (kk - cc + 2048).astype(np.float64)
    xslope = (2.0 ** 8) ** (1.0 / H)
    tables = []
    for r in range(W):
        heads = []
        for hh in range(HPC):
            h = 2 * r + hh
            slope = 1.0 / xslope ** (h + 1)
            with np.errstate(over="ignore"):
                g = np.where(rel <= 0, np.exp(slope * rel), 0.0)
            heads.append(g.astype(np.float32))
        tables.append(np.stack(heads).astype(ml_dtypes.bfloat16))
    return tables


def kernel(x, w_kqv, ln1_g, ln1_b, ln2_g, ln2_b, w1, b1, w2, b2, _debug=False):
    x = np.asarray(x, np.float32)
    x_flat = np.ascontiguousarray(x.reshape(T, E))
    wk = np.asarray(w_kqv, np.float32)
    bf = ml_dtypes.bfloat16
    w1_bf = np.ascontiguousarray(np.asarray(w1, np.float32).astype(bf))
    w2_bf = np.ascontiguousarray(np.asarray(w2, np.float32).astype(bf))

    key = "nc_dbg" if _debug else "nc"
    if key not in _CACHE:
        _CACHE[key] = _build(debug=_debug)
        _CACHE.setdefault("expb", _expbias_table())
    nc = _CACHE[key]
    expbs = _CACHE["expb"]

    in_maps = []
    for r in range(W):
        cols = slice(128 * r, 128 * (r + 1))
        wqk_r = np.concatenate([wk[:, 0:E][:, cols], wk[:, E:2 * E][:, cols]],
                               axis=1).astype(bf)
        wv_r = np.ascontiguousarray(wk[:, 2 * E:3 * E][:, cols]).astype(bf)
        in_maps.append({
            "x_s": np.ascontiguousarray(x_flat[TPC * r:TPC * (r + 1)]),
            "wqk": np.ascontiguousarray(wqk_r),
            "wv": wv_r,
            "w1b": w1_bf,
            "w2b": w2_bf,
            "b1": np.ascontiguousarray(np.asarray(b1, np.float32)),
            "b2": np.ascontiguousarray(np.asarray(b2, np.float32)),
            "g1": np.ascontiguousarray(np.asarray(ln1_g, np.float32)),
            "be1": np.ascontiguousarray(np.asarray(ln1_b, np.float32)),
            "g2": np.ascontiguousarray(np.asarray(ln2_g, np.float32)),
            "be2": np.ascontiguousarray(np.asarray(ln2_b, np.float32)),
            "expb": expbs[r],
        })

    _CACHE["last_in_maps"] = in_maps
    res = run_bass_kernel_spmd(nc, in_maps, core_ids=list(range(W)))
    out = np.concatenate([res.results[r]["out"] for r in range(W)], axis=0)
    if _debug:
        return out.reshape(B, S, E), res.results
    return out.reshape(B, S, E)


if __name__ == "__main__":
    import reference
    inputs = {k: np.asarray(v) for k, v in reference.setup_inputs().items()}
    got = kernel(**inputs)
    exp = np.asarray(reference.reference(**reference.setup_inputs()))
    err = np.abs(got - exp).max() / np.abs(exp).max()
    print("Relative error:", err)


# revision 13
# speedup vs baseline: 1.2022x; 1.1158x over previous
"""ALiBi transformer layer on 8 TRN2 NeuronCores.

Sharding: tensor-parallel attention (16 heads -> 2 per core) +
sequence-parallel FFN (4096 tokens -> 512 per core, full w1/w2 replicated).
Collectives: a tiny prewarm AllGather (absorbs the NEFF-entry rank-skew
barrier), a 2-way-split bf16 AllGather of ln1(x)^T (per-rank 0.5MB each,
lets kqv start after the first half lands), and one bf16 AllToAll of
per-head attention outputs.

Matmul compute in bf16 (fp32 accumulation in PSUM); layernorms, softmax
normalization and residuals in fp32. Softmax uses a fixed max-shift
(exp(s/32 - 4)) which is safe because |scores| <= ~1 here, and the
ALiBi+causal term is folded in as a host-precomputed multiplicative
table exp(slope*(j-i)) (0 where masked), so no max-reduction and no
transposes of the attention matrix are needed. The softmax denominator
comes free from an appended ones-column in the V operand.
"""

import sys

sys.path.insert(0, "/opt/trn_rl_repo")

import math

import ml_dtypes
import numpy as np

import concourse.bass as bass
import concourse.tile as tile
from concourse import bacc, mybir
from concourse.bass_utils import run_bass_kernel_spmd
from concourse.masks import make_identity

F32 = mybir.dt.float32
BF16 = mybir.dt.bfloat16
AF = mybir.ActivationFunctionType

B, S, E = 2, 2048, 1024
H, D = 16, 64            # heads, head dim
DFF = 4096
W = 8                    # cores
T = B * S                # 4096 tokens
TPC = T // W             # 512 tokens per core
HPC = H // W             # 2 heads per core
EPS = 1e-5
SCALE = 1.0 / math.sqrt(E)
EXP_SHIFT = -4.0         # fixed softmax shift; scores are << 4 here
RG = [list(range(W))]

_CACHE = {}


def _ln(nc, pools, x_ap, g_b, b_b, out_bf, eps_t):
    """LayerNorm over free dim (1024) of x_ap [128, 1024] f32 -> out_bf bf16."""
    stats = pools.tile([128, 2, 6], F32, tag="ln_stats")
    xg = x_ap.rearrange("p (s f) -> p s f", s=2)
    for sg in range(2):
        nc.vector.bn_stats(out=stats[:, sg, :], in_=xg[:, sg, :])
    mv = pools.tile([128, 2], F32, tag="ln_mv")
    nc.vector.bn_aggr(out=mv[:], in_=stats[:])
    nc.scalar.activation(out=mv[:, 1:2], in_=mv[:, 1:2], func=AF.Sqrt,
                         bias=eps_t[:], scale=1.0)
    nc.vector.reciprocal(out=mv[:, 1:2], in_=mv[:, 1:2])
    xc = pools.tile([128, 1024], F32, tag="ln_xc")
    nc.vector.tensor_scalar(out=xc[:], in0=x_ap, scalar1=mv[:, 0:1],
                            scalar2=mv[:, 1:2],
                            op0=mybir.AluOpType.subtract,
                            op1=mybir.AluOpType.mult)
    nc.vector.tensor_mul(out=xc[:], in0=xc[:], in1=g_b[:])
    nc.vector.tensor_add(out=out_bf, in0=xc[:], in1=b_b[:])


def _build(debug=False):
    nc = bacc.Bacc(None, target_bir_lowering=False, num_devices=W)

    x_s = nc.declare_dram_parameter("x_s", [TPC, E], F32, isOutput=False)
    wqk = nc.declare_dram_parameter("wqk", [E, 2 * HPC * D], BF16, isOutput=False)
    wv = nc.declare_dram_parameter("wv", [E, HPC * D], BF16, isOutput=False)
    w1b = nc.declare_dram_parameter("w1b", [E, DFF], BF16, isOutput=False)
    w2b = nc.declare_dram_parameter("w2b", [DFF, E], BF16, isOutput=False)
    b1_d = nc.declare_dram_parameter("b1", [DFF], F32, isOutput=False)
    b2_d = nc.declare_dram_parameter("b2", [E], F32, isOutput=False)
    g1_d = nc.declare_dram_parameter("g1", [E], F32, isOutput=False)
    be1_d = nc.declare_dram_parameter("be1", [E], F32, isOutput=False)
    g2_d = nc.declare_dram_parameter("g2", [E], F32, isOutput=False)
    be2_d = nc.declare_dram_parameter("be2", [E], F32, isOutput=False)
    expb_d = nc.declare_dram_parameter("expb", [HPC, 128, 4096], BF16, isOutput=False)
    out_ext = nc.declare_dram_parameter("out", [TPC, E], F32, isOutput=True)
    if debug:
        dbg_attn = nc.declare_dram_parameter("dbg_attn", [128, 8, 4, 128], BF16,
                                             isOutput=True)

    def bcast_row(dram_ap):
        return bass.AP(tensor=dram_ap.tensor, offset=0, ap=[[0, 128], [1, E]])

    with tile.TileContext(nc) as tc:
        with (
            tc.tile_pool(name="dram", bufs=1, space="DRAM") as dram,
            tc.tile_pool(name="params", bufs=1) as prm,
            tc.tile_pool(name="persist", bufs=1) as per,
            tc.tile_pool(name="lntmp", bufs=2) as lnp,
        ):
            # ---- DRAM bounce buffers for collectives ----
            ag_in = dram.tile([E, TPC], BF16)
            ag_out = dram.tile([W * E, TPC], BF16, addr_space="Shared")
            a2a_in = dram.tile([T, HPC * D], BF16)
            a2a_out = dram.tile([T, HPC * D], BF16)

            # ---- critical-path params first ----
            g1_b = prm.tile([128, E], F32)
            be1_b = prm.tile([128, E], F32)
            nc.sync.dma_start(out=g1_b[:], in_=bcast_row(g1_d[:]))
            nc.sync.dma_start(out=be1_b[:], in_=bcast_row(be1_d[:]))
            eps_t = prm.tile([128, 1], F32)
            nc.vector.memset(eps_t[:], EPS)
            ident_bf = prm.tile([128, 128], BF16)
            make_identity(nc, ident_bf[:])

            # ---- persistent activations ----
            x_sb = per.tile([128, 4, E], F32)        # x, then x+attn in place
            xb2_sb = per.tile([128, 4, E], F32)      # x+attn+b2 (fc2 epilogue)
            qT_sb = per.tile([128, 8, 512], BF16)    # [qk-col(2h*64), chunk, tok]
            kT_sb = per.tile([128, 8, 512], BF16)
            v_sb = per.tile([128, 8, 4, 2 * (D + 1)], BF16)  # v + ones cols
            h2T_sb = per.tile([128, 8, 512], BF16)
            aT_sb = per.tile([128, 32, 512], BF16)
            attn_sb = per.tile([128, 8, 4, 128], BF16)

            nc.vector.memset(v_sb[:], 1.0)

            # =========== Phase 1: LN1 + transpose + split AllGather ===========
            with (
                tc.tile_pool(name="h1stage", bufs=1) as h1s,
                tc.tile_pool(name="pT", bufs=4, space="PSUM") as pT,
            ):
                h1T_sb = h1s.tile([128, 8, 512], BF16)
                for tt in range(4):
                    for q4 in range(4):
                        nc.sync.dma_start(
                            out=x_sb[:, tt, 256 * q4:256 * (q4 + 1)],
                            in_=x_s[128 * tt:128 * (tt + 1), 256 * q4:256 * (q4 + 1)])
                    h1_bf = lnp.tile([128, E], BF16, tag="h1bf")
                    _ln(nc, lnp, x_sb[:, tt, :], g1_b, be1_b, h1_bf[:], eps_t)
                    for eb in range(8):
                        pt = pT.tile([128, 128], BF16)
                        nc.tensor.transpose(pt[:], h1_bf[:, 128 * eb:128 * (eb + 1)],
                                            ident_bf[:])
                        if eb % 2:
                            nc.scalar.copy(out=h1T_sb[:, eb, 128 * tt:128 * (tt + 1)],
                                           in_=pt[:])
                        else:
                            nc.vector.tensor_copy(
                                out=h1T_sb[:, eb, 128 * tt:128 * (tt + 1)], in_=pt[:])
                nc.sync.dma_start(
                    out=ag_in[:].rearrange("(eb p) t -> p eb t", p=128),
                    in_=h1T_sb[:])
                nc.gpsimd.collective_compute(
                    "AllGather", mybir.AluOpType.bypass, replica_groups=RG,
                    ins=[ag_in.opt()], outs=[ag_out.opt()])

            # ---- remaining params (off the phase-1 critical path) ----
            g2_b = prm.tile([128, E], F32)
            be2_b = prm.tile([128, E], F32)
            b2_b = prm.tile([128, E], F32)
            for t_, d_ in ((g2_b, g2_d), (be2_b, be2_d), (b2_b, b2_d)):
                nc.sync.dma_start(out=t_[:], in_=bcast_row(d_[:]))
            b1_sb = prm.tile([128, DFF // 128], F32)
            nc.sync.dma_start(out=b1_sb[:], in_=b1_d[:].rearrange("(j p) -> p j", p=128))
            neg4 = prm.tile([128, 1], F32)
            nc.vector.memset(neg4[:], EXP_SHIFT)
            ident_f = prm.tile([128, 128], F32)
            make_identity(nc, ident_f[:])
            expb_sb = prm.tile([128, HPC, 4096], BF16)
            nc.sync.dma_start(out=expb_sb[:],
                              in_=expb_d[:].rearrange("h p c -> p h c"))
            wqk_sb = prm.tile([128, 8, 2 * HPC * D], BF16)
            nc.sync.dma_start(out=wqk_sb[:],
                              in_=wqk[:].rearrange("(eb p) c -> p eb c", p=128))
            wv_sb = prm.tile([128, 8, HPC * D], BF16)
            nc.sync.dma_start(out=wv_sb[:],
                              in_=wv[:].rearrange("(eb p) c -> p eb c", p=128))

            ag_v = ag_out[:].rearrange("(r eb p) t -> r eb p t", eb=8, p=128)

            # ====== Phases 2+3: kqv interleaved with attention ======
            # kqv chunks are PE-dense, attention is ACT/DVE-dense; interleaving
            # keeps the PE busy enough that HAM holds the 2.4 GHz clock.
            # Attention processes both heads per (gq, kb): the two K=64 QK
            # matmuls run concurrently in PE row-groups 0 and 64, and one
            # exp + one expbias-multiply cover both heads' scores.
            with (
                tc.tile_pool(name="h1t", bufs=2) as h1tp,
                tc.tile_pool(name="pqa", bufs=1, space="PSUM") as pqa,
                tc.tile_pool(name="att_t", bufs=4) as att,
            ):
                def kqv_chunk(c):
                    pk = pqa.tile([128, 512], F32, name="pk", tag="pk", bufs=1)
                    pq = pqa.tile([128, 512], F32, name="pq", tag="pq", bufs=1)
                    h1ts = []
                    for eb in range(8):
                        h1t = h1tp.tile([128, 512], BF16, name=f"h1t{eb}")
                        nc.sync.dma_start(out=h1t[:], in_=ag_v[c, eb])
                        h1ts.append(h1t)
                        st, sp = eb == 0, eb == 7
                        nc.tensor.matmul(pk[:], wqk_sb[:, eb, 0:128], h1t[:],
                                         start=st, stop=sp)
                        nc.tensor.matmul(pq[:], wqk_sb[:, eb, 128:256], h1t[:],
                                         start=st, stop=sp)
                    nc.vector.tensor_copy(out=kT_sb[:, c, :], in_=pk[:])
                    nc.scalar.copy(out=qT_sb[:, c, :], in_=pq[:])
                    for tb in range(4):
                        # own psum bank per tb: start=True clears a whole bank
                        pv = pqa.tile([128, 128], F32, name="pv", tag="small",
                                      bufs=2)
                        for eb in range(8):
                            nc.tensor.matmul(
                                pv[:], h1ts[eb][:, 128 * tb:128 * (tb + 1)],
                                wv_sb[:, eb, :], start=(eb == 0), stop=(eb == 7))
                        for hh in range(HPC):
                            if (tb + hh) % 2:
                                nc.vector.tensor_copy(
                                    out=v_sb[:, c, tb, 65 * hh:65 * hh + 64],
                                    in_=pv[:, 64 * hh:64 * hh + 64])
                            else:
                                nc.scalar.copy(
                                    out=v_sb[:, c, tb, 65 * hh:65 * hh + 64],
                                    in_=pv[:, 64 * hh:64 * hh + 64])

                def attn_group(gq):
                    b_, qc = gq // 4, gq % 4
                    nkb = 4 * (qc + 1)
                    po0 = pqa.tile([65, 512], F32, name="po0", tag="po0", bufs=1)
                    po1 = pqa.tile([65, 512], F32, name="po1", tag="po1", bufs=1)
                    pos_ = (po0, po1)
                    for kb in range(nkb):
                        ck, tbk = 4 * b_ + kb // 4, kb % 4
                        ps = pqa.tile([128, 2, 512], F32, name="ps", tag="ps",
                                      bufs=1)
                        for hh in range(HPC):
                            nc.tensor.matmul(
                                ps[:, hh, :],
                                kT_sb[64 * hh:64 * hh + 64, ck,
                                      128 * tbk:128 * (tbk + 1)],
                                qT_sb[64 * hh:64 * hh + 64, gq, :],
                                start=True, stop=True,
                                tile_position=(64 * hh, 0))
                        e_bf = att.tile([128, 2, 512], BF16, tag="ebf")
                        nc.scalar.activation(
                            out=e_bf[:].rearrange("p a b -> p (a b)"),
                            in_=ps[:].rearrange("p a b -> p (a b)"),
                            func=AF.Exp, bias=neg4[:], scale=SCALE)
                        p_bf = att.tile([128, 2, 512], BF16, tag="pbf")
                        off = 2048 - (128 * kb - 512 * qc)
                        nc.vector.tensor_mul(out=p_bf[:], in0=e_bf[:],
                                             in1=expb_sb[:, :, off:off + 512])
                        for hh in range(HPC):
                            nc.tensor.matmul(
                                pos_[hh][:],
                                v_sb[:, ck, tbk, 65 * hh:65 * (hh + 1)],
                                p_bf[:, hh, :],
                                start=(kb == 0), stop=(kb == nkb - 1))
                    for hh in range(HPC):
                        o_sb = att.tile([65, 512], F32, tag="osb")
                        if hh:
                            nc.scalar.copy(out=o_sb[:], in_=pos_[hh][:])
                        else:
                            nc.vector.tensor_copy(out=o_sb[:], in_=pos_[hh][:])
                        for tb in range(4):
                            pt = pqa.tile([128, 65], F32, name="pt", tag="small",
                                          bufs=2)
                            nc.tensor.transpose(pt[:],
                                                o_sb[:, 128 * tb:128 * (tb + 1)],
                                                ident_f[:65, :65])
                            rc = att.tile([128, 1], F32, tag="rc")
                            nc.vector.reciprocal(out=rc[:], in_=pt[:, 64:65])
                            dst = attn_sb[:, gq, tb, 64 * hh:64 * (hh + 1)]
                            if tb % 2:
                                nc.scalar.activation(out=dst, in_=pt[:, 0:64],
                                                     func=AF.Copy, scale=rc[:])
                            else:
                                nc.vector.tensor_scalar_mul(out=dst,
                                                            in0=pt[:, 0:64],
                                                            scalar1=rc[:])
                    nc.sync.dma_start(
                        out=a2a_in[:].rearrange("(g tb p) c -> g p tb c",
                                                tb=4, p=128)[gq],
                        in_=attn_sb[:, gq, :, :])

                # snake order spreads the PE-dense kqv chunks across the
                # ACT/DVE-dense attention groups (deps: attn(gq) needs chunks
                # 4*(gq//4) .. gq)
                kqv_chunk(0)
                kqv_chunk(4)
                feed = [1, 5, 2, 6, 3, 7]
                for i, gq in enumerate([0, 4, 1, 5, 2, 6, 3, 7]):
                    attn_group(gq)
                    if i < len(feed):
                        kqv_chunk(feed[i])
            if debug:
                nc.sync.dma_start(out=dbg_attn[:], in_=attn_sb[:])
            nc.gpsimd.collective_compute(
                "AllToAll", mybir.AluOpType.bypass, replica_groups=RG,
                ins=[a2a_in.opt()], outs=[a2a_out.opt()])

            a2a_v = a2a_out[:].rearrange("(r tk p) c -> tk p r c", r=W, p=128)

            # =========== Phase 4: residual + LN2 + transpose ===========
            with (
                tc.tile_pool(name="afull", bufs=2) as afp,
                tc.tile_pool(name="pT2", bufs=4, space="PSUM") as pT2,
            ):
                for tt in range(4):
                    af = afp.tile([128, W, 128], BF16)
                    nc.sync.dma_start(out=af[:], in_=a2a_v[tt])
                    nc.vector.tensor_add(out=x_sb[:, tt, :], in0=x_sb[:, tt, :],
                                         in1=af[:].rearrange("p r c -> p (r c)"))
                    nc.vector.tensor_add(out=xb2_sb[:, tt, :], in0=x_sb[:, tt, :],
                                         in1=b2_b[:])
                    h2_bf = lnp.tile([128, E], BF16, tag="h2bf")
                    _ln(nc, lnp, x_sb[:, tt, :], g2_b, be2_b, h2_bf[:], eps_t)
                    for eb in range(8):
                        pt = pT2.tile([128, 128], BF16)
                        nc.tensor.transpose(pt[:], h2_bf[:, 128 * eb:128 * (eb + 1)],
                                            ident_bf[:])
                        if eb % 2:
                            nc.scalar.copy(out=h2T_sb[:, eb, 128 * tt:128 * (tt + 1)],
                                           in_=pt[:])
                        else:
                            nc.vector.tensor_copy(
                                out=h2T_sb[:, eb, 128 * tt:128 * (tt + 1)], in_=pt[:])

            # =========== Phase 5: fc1 (a^T = relu(w1^T h2^T + b1)) ===========
            with (
                tc.tile_pool(name="w1t", bufs=6) as w1p,
                tc.tile_pool(name="ps_a", bufs=1, space="PSUM") as ps_a,
            ):
                for dffc in range(8):
                    pa = [ps_a.tile([128, 512], F32, name=f"pa{i}", tag=f"pa{i}")
                          for i in range(4)]
                    for eb in range(8):
                        w1t = w1p.tile([128, 512], BF16)
                        nc.sync.dma_start(
                            out=w1t[:],
                            in_=w1b[128 * eb:128 * (eb + 1),
                                    512 * dffc:512 * (dffc + 1)])
                        for db in range(4):
                            nc.tensor.matmul(pa[db][:],
                                             w1t[:, 128 * db:128 * (db + 1)],
                                             h2T_sb[:, eb, :],
                                             start=(eb == 0), stop=(eb == 7))
                    for db in range(4):
                        j = 4 * dffc + db
                        nc.scalar.activation(out=aT_sb[:, j, :], in_=pa[db][:],
                                             func=AF.Relu, bias=b1_sb[:, j:j + 1],
                                             scale=1.0)

            # =========== Phase 6: fc2 + residual ===========
            with (
                tc.tile_pool(name="w2t", bufs=6) as w2p,
                tc.tile_pool(name="ps_f", bufs=1, space="PSUM") as ps_f,
                tc.tile_pool(name="outp", bufs=4) as outp,
            ):
                pf = [[ps_f.tile([128, 512], F32, name=f"pf{i}_{j}", tag=f"pf{i}_{j}")
                       for j in range(2)] for i in range(4)]
                for db in range(32):
                    w2t = w2p.tile([128, E], BF16)
                    nc.sync.dma_start(out=w2t[:], in_=w2b[128 * db:128 * (db + 1), :])
                    for tt in range(4):
                        for eh in range(2):
                            nc.tensor.matmul(pf[tt][eh][:],
                                             aT_sb[:, db, 128 * tt:128 * (tt + 1)],
                                             w2t[:, 512 * eh:512 * (eh + 1)],
                                             start=(db == 0), stop=(db == 31))
                for tt in range(4):
                    for eh in range(2):
                        ot = outp.tile([128, 512], F32)
                        nc.vector.tensor_add(
                            out=ot[:], in0=pf[tt][eh][:],
                            in1=xb2_sb[:, tt, 512 * eh:512 * (eh + 1)])
                        nc.sync.dma_start(
                            out=out_ext[128 * tt:128 * (tt + 1),
                                        512 * eh:512 * (eh + 1)],
                            in_=ot[:])
    nc.compile()
    return nc


def _expbias_table():
    """[HPC, 128, 4096] per-core list: exp(slope*(k - c + 2048)), 0 where future."""
    kk = np.arange(128)[:, None]
    cc = np.arange(4096)[None, :]
    rel = (kk - cc + 2048).astype(np.float64)
    xslope = (2.0 ** 8) ** (1.0 / H)
    tables = []
    for r in range(W):
        heads = []
        for hh in range(HPC):
            h = 2 * r + hh
            slope = 1.0 / xslope ** (h + 1)
            with np.errstate(over="ignore"):
                g = np.where(rel <= 0, np.exp(slope * rel), 0.0)
            heads.append(g.astype(np.float32))
        tables.append(np.stack(heads).astype(ml_dtypes.bfloat16))
    return tables


def kernel(x, w_kqv, ln1_g, ln1_b, ln2_g, ln2_b, w1, b1, w2, b2, _debug=False):
    x = np.asarray(x, np.float32)
    x_flat = np.ascontiguousarray(x.reshape(T, E))
    wk = np.asarray(w_kqv, np.float32)
    bf = ml_dtypes.bfloat16
    w1_bf = np.ascontiguousarray(np.asarray(w1, np.float32).astype(bf))
    w2_bf = np.ascontiguousarray(np.asarray(w2, np.float32).astype(bf))

    key = "nc_dbg" if _debug else "nc"
    if key not in _CACHE:
        _CACHE[key] = _build(debug=_debug)
        _CACHE.setdefault("expb", _expbias_table())
    nc = _CACHE[key]
    expbs = _CACHE["expb"]

    in_maps = []
    for r in range(W):
        cols = slice(128 * r, 128 * (r + 1))
        wqk_r = np.concatenate([wk[:, 0:E][:, cols], wk[:, E:2 * E][:, cols]],
                               axis=1).astype(bf)
        wv_r = np.ascontiguousarray(wk[:, 2 * E:3 * E][:, cols]).astype(bf)
        in_maps.append({
            "x_s": np.ascontiguousarray(x_flat[TPC * r:TPC * (r + 1)]),
            "wqk": np.ascontiguousarray(wqk_r),
            "wv": wv_r,
            "w1b": w1_bf,
            "w2b": w2_bf,
            "b1": np.ascontiguousarray(np.asarray(b1, np.float32)),
            "b2": np.ascontiguousarray(np.asarray(b2, np.float32)),
            "g1": np.ascontiguousarray(np.asarray(ln1_g, np.float32)),
            "be1": np.ascontiguousarray(np.asarray(ln1_b, np.float32)),
            "g2": np.ascontiguousarray(np.asarray(ln2_g, np.float32)),
            "be2": np.ascontiguousarray(np.asarray(ln2_b, np.float32)),
            "expb": expbs[r],
        })

    _CACHE["last_in_maps"] = in_maps
    res = run_bass_kernel_spmd(nc, in_maps, core_ids=list(range(W)))
    out = np.concatenate([res.results[r]["out"] for r in range(W)], axis=0)
    if _debug:
        return out.reshape(B, S, E), res.results
    return out.reshape(B, S, E)


if __name__ == "__main__":
    import reference
    inputs = {k: np.asarray(v) for k, v in reference.setup_inputs().items()}
    got = kernel(**inputs)
    exp = np.asarray(reference.reference(**reference.setup_inputs()))
    err = np.abs(got - exp).max() / np.abs(exp).max()
    print("Relative error:", err)


# revision 15
# speedup vs baseline: 1.2422x; 1.0333x over previous
"""ALiBi transformer layer on 8 TRN2 NeuronCores.

Sharding: tensor-parallel attention (16 heads -> 2 per core) +
sequence-parallel FFN (4096 tokens -> 512 per core, full w1/w2 replicated).
Collectives: a tiny prewarm AllGather (absorbs the NEFF-entry rank-skew
barrier), a 2-way-split bf16 AllGather of ln1(x)^T (per-rank 0.5MB each,
lets kqv start after the first half lands), and one bf16 AllToAll of
per-head attention outputs.

Matmul compute in bf16 (fp32 accumulation in PSUM); layernorms, softmax
normalization and residuals in fp32. Softmax uses a fixed max-shift
(exp(s/32 - 4)) which is safe because |scores| <= ~1 here, and the
ALiBi+causal term is folded in as a host-precomputed multiplicative
table exp(slope*(j-i)) (0 where masked), so no max-reduction and no
transposes of the attention matrix are needed. The softmax denominator
comes free from an appended ones-column in the V operand.
"""

import sys

sys.path.insert(0, "/opt/trn_rl_repo")

import math

import ml_dtypes
import numpy as np

import concourse.bass as bass
import concourse.tile as tile
from concourse import bacc, mybir
from concourse.bass_utils import run_bass_kernel_spmd
from concourse.masks import make_identity

F32 = mybir.dt.float32
BF16 = mybir.dt.bfloat16
AF = mybir.ActivationFunctionType

B, S, E = 2, 2048, 1024
H, D = 16, 64            # heads, head dim
DFF = 4096
W = 8                    # cores
T = B * S                # 4096 tokens
TPC = T // W             # 512 tokens per core
HPC = H // W             # 2 heads per core
EPS = 1e-5
SCALE = 1.0 / math.sqrt(E)
EXP_SHIFT = -4.0         # fixed softmax shift; scores are << 4 here
RG = [list(range(W))]

_CACHE = {}


def _ln(nc, pools, x_ap, g_b, b_b, out_bf, eps_t):
    """LayerNorm over free dim (1024) of x_ap [128, 1024] f32 -> out_bf bf16."""
    stats = pools.tile([128, 2, 6], F32, tag="ln_stats")
    xg = x_ap.rearrange("p (s f) -> p s f", s=2)
    for sg in range(2):
        nc.vector.bn_stats(out=stats[:, sg, :], in_=xg[:, sg, :])
    mv = pools.tile([128, 2], F32, tag="ln_mv")
    nc.vector.bn_aggr(out=mv[:], in_=stats[:])
    nc.scalar.activation(out=mv[:, 1:2], in_=mv[:, 1:2], func=AF.Sqrt,
                         bias=eps_t[:], scale=1.0)
    nc.vector.reciprocal(out=mv[:, 1:2], in_=mv[:, 1:2])
    xc = pools.tile([128, 1024], F32, tag="ln_xc")
    nc.vector.tensor_scalar(out=xc[:], in0=x_ap, scalar1=mv[:, 0:1],
                            scalar2=mv[:, 1:2],
                            op0=mybir.AluOpType.subtract,
                            op1=mybir.AluOpType.mult)
    nc.vector.tensor_mul(out=xc[:], in0=xc[:], in1=g_b[:])
    nc.vector.tensor_add(out=out_bf, in0=xc[:], in1=b_b[:])


def _build(debug=False):
    nc = bacc.Bacc(None, target_bir_lowering=False, num_devices=W)

    x_s = nc.declare_dram_parameter("x_s", [TPC, E], F32, isOutput=False)
    wqk = nc.declare_dram_parameter("wqk", [E, 2 * HPC * D], BF16, isOutput=False)
    wv = nc.declare_dram_parameter("wv", [E, HPC * D], BF16, isOutput=False)
    w1b = nc.declare_dram_parameter("w1b", [E, DFF], BF16, isOutput=False)
    w2b = nc.declare_dram_parameter("w2b", [DFF, E], BF16, isOutput=False)
    b1_d = nc.declare_dram_parameter("b1", [DFF], F32, isOutput=False)
    b2_d = nc.declare_dram_parameter("b2", [E], F32, isOutput=False)
    g1_d = nc.declare_dram_parameter("g1", [E], F32, isOutput=False)
    be1_d = nc.declare_dram_parameter("be1", [E], F32, isOutput=False)
    g2_d = nc.declare_dram_parameter("g2", [E], F32, isOutput=False)
    be2_d = nc.declare_dram_parameter("be2", [E], F32, isOutput=False)
    expb_d = nc.declare_dram_parameter("expb", [HPC, 128, 4096], BF16, isOutput=False)
    out_ext = nc.declare_dram_parameter("out", [TPC, E], F32, isOutput=True)
    if debug:
        dbg_attn = nc.declare_dram_parameter("dbg_attn", [128, 8, 4, 128], BF16,
                                             isOutput=True)

    def bcast_row(dram_ap):
        return bass.AP(tensor=dram_ap.tensor, offset=0, ap=[[0, 128], [1, E]])

    with tile.TileContext(nc) as tc:
        with (
            tc.tile_pool(name="dram", bufs=1, space="DRAM") as dram,
            tc.tile_pool(name="params", bufs=1) as prm,
            tc.tile_pool(name="persist", bufs=1) as per,
            tc.tile_pool(name="lntmp", bufs=2) as lnp,
        ):
            # ---- DRAM bounce buffers for collectives ----
            ag_in = dram.tile([E, TPC], BF16)
            ag_out = dram.tile([W * E, TPC], BF16, addr_space="Shared")
            a2a_in = dram.tile([T, HPC * D], BF16)
            a2a_out = dram.tile([T, HPC * D], BF16)

            # ---- critical-path params first ----
            g1_b = prm.tile([128, E], F32)
            be1_b = prm.tile([128, E], F32)
            nc.sync.dma_start(out=g1_b[:], in_=bcast_row(g1_d[:]))
            nc.sync.dma_start(out=be1_b[:], in_=bcast_row(be1_d[:]))
            eps_t = prm.tile([128, 1], F32)
            nc.vector.memset(eps_t[:], EPS)
            ident_bf = prm.tile([128, 128], BF16)
            make_identity(nc, ident_bf[:])

            # ---- persistent activations ----
            x_sb = per.tile([128, 4, E], F32)        # x, then x+attn in place
            xb2_sb = per.tile([128, 4, E], F32)      # x+attn+b2 (fc2 epilogue)
            qT_sb = per.tile([128, 8, 512], BF16)    # [qk-col(2h*64), chunk, tok]
            kT_sb = per.tile([128, 8, 512], BF16)
            v_sb = per.tile([128, 8, 4, 2 * (D + 1)], BF16)  # v + ones cols
            h2T_sb = per.tile([128, 8, 512], BF16)
            aT_sb = per.tile([128, 32, 512], BF16)
            attn_sb = per.tile([128, 8, 4, 128], BF16)

            nc.vector.memset(v_sb[:], 1.0)

            # =========== Phase 1: LN1 + transpose + split AllGather ===========
            with (
                tc.tile_pool(name="h1stage", bufs=1) as h1s,
                tc.tile_pool(name="pT", bufs=4, space="PSUM") as pT,
            ):
                h1T_sb = h1s.tile([128, 8, 512], BF16)
                for tt in range(4):
                    for q4 in range(4):
                        nc.sync.dma_start(
                            out=x_sb[:, tt, 256 * q4:256 * (q4 + 1)],
                            in_=x_s[128 * tt:128 * (tt + 1), 256 * q4:256 * (q4 + 1)])
                    h1_bf = lnp.tile([128, E], BF16, tag="h1bf")
                    _ln(nc, lnp, x_sb[:, tt, :], g1_b, be1_b, h1_bf[:], eps_t)
                    for eb in range(8):
                        pt = pT.tile([128, 128], BF16)
                        nc.tensor.transpose(pt[:], h1_bf[:, 128 * eb:128 * (eb + 1)],
                                            ident_bf[:])
                        if eb % 2:
                            nc.scalar.copy(out=h1T_sb[:, eb, 128 * tt:128 * (tt + 1)],
                                           in_=pt[:])
                        else:
                            nc.vector.tensor_copy(
                                out=h1T_sb[:, eb, 128 * tt:128 * (tt + 1)], in_=pt[:])
                agv_in = ag_in[:].rearrange("(eb p) t -> p eb t", p=128)
                for tt in range(4):
                    nc.sync.dma_start(
                        out=agv_in[:, :, 128 * tt:128 * (tt + 1)],
                        in_=h1T_sb[:, :, 128 * tt:128 * (tt + 1)])
                nc.gpsimd.collective_compute(
                    "AllGather", mybir.AluOpType.bypass, replica_groups=RG,
                    ins=[ag_in.opt()], outs=[ag_out.opt()])

            # ---- remaining params (off the phase-1 critical path) ----
            g2_b = prm.tile([128, E], F32)
            be2_b = prm.tile([128, E], F32)
            b2_b = prm.tile([128, E], F32)
            for t_, d_ in ((g2_b, g2_d), (be2_b, be2_d), (b2_b, b2_d)):
                nc.sync.dma_start(out=t_[:], in_=bcast_row(d_[:]))
            b1_sb = prm.tile([128, DFF // 128], F32)
            nc.sync.dma_start(out=b1_sb[:], in_=b1_d[:].rearrange("(j p) -> p j", p=128))
            neg4 = prm.tile([128, 1], F32)
            nc.vector.memset(neg4[:], EXP_SHIFT)
            ident_f = prm.tile([128, 128], F32)
            make_identity(nc, ident_f[:])
            expb_sb = prm.tile([128, HPC, 4096], BF16)
            nc.sync.dma_start(out=expb_sb[:],
                              in_=expb_d[:].rearrange("h p c -> p h c"))
            wqk_sb = prm.tile([128, 8, 2 * HPC * D], BF16)
            nc.sync.dma_start(out=wqk_sb[:],
                              in_=wqk[:].rearrange("(eb p) c -> p eb c", p=128))
            wv_sb = prm.tile([128, 8, HPC * D], BF16)
            nc.sync.dma_start(out=wv_sb[:],
                              in_=wv[:].rearrange("(eb p) c -> p eb c", p=128))

            ag_v = ag_out[:].rearrange("(r eb p) t -> r eb p t", eb=8, p=128)

            # ====== Phases 2+3: kqv interleaved with attention ======
            # kqv chunks are PE-dense, attention is ACT/DVE-dense; interleaving
            # keeps the PE busy enough that HAM holds the 2.4 GHz clock.
            # Attention processes both heads per (gq, kb): the two K=64 QK
            # matmuls run concurrently in PE row-groups 0 and 64, and one
            # exp + one expbias-multiply cover both heads' scores.
            with (
                tc.tile_pool(name="h1t", bufs=2) as h1tp,
                tc.tile_pool(name="pqa", bufs=1, space="PSUM") as pqa,
                tc.tile_pool(name="att_t", bufs=4) as att,
            ):
                def kqv_chunk(c):
                    pk = pqa.tile([128, 512], F32, name="pk", tag="pk", bufs=1)
                    pq = pqa.tile([128, 512], F32, name="pq", tag="pq", bufs=1)
                    h1ts = []
                    for eb in range(8):
                        h1t = h1tp.tile([128, 512], BF16, name=f"h1t{eb}")
                        nc.sync.dma_start(out=h1t[:], in_=ag_v[c, eb])
                        h1ts.append(h1t)
                        st, sp = eb == 0, eb == 7
                        nc.tensor.matmul(pk[:], wqk_sb[:, eb, 0:128], h1t[:],
                                         start=st, stop=sp)
                        nc.tensor.matmul(pq[:], wqk_sb[:, eb, 128:256], h1t[:],
                                         start=st, stop=sp)
                    nc.vector.tensor_copy(out=kT_sb[:, c, :], in_=pk[:])
                    nc.scalar.copy(out=qT_sb[:, c, :], in_=pq[:])
                    for tb in range(4):
                        # own psum bank per tb: start=True clears a whole bank
                        pv = pqa.tile([128, 128], F32, name="pv", tag="small",
                                      bufs=2)
                        for eb in range(8):
                            nc.tensor.matmul(
                                pv[:], h1ts[eb][:, 128 * tb:128 * (tb + 1)],
                                wv_sb[:, eb, :], start=(eb == 0), stop=(eb == 7))
                        for hh in range(HPC):
                            if (tb + hh) % 2:
                                nc.vector.tensor_copy(
                                    out=v_sb[:, c, tb, 65 * hh:65 * hh + 64],
                                    in_=pv[:, 64 * hh:64 * hh + 64])
                            else:
                                nc.scalar.copy(
                                    out=v_sb[:, c, tb, 65 * hh:65 * hh + 64],
                                    in_=pv[:, 64 * hh:64 * hh + 64])

                def attn_group(gq):
                    b_, qc = gq // 4, gq % 4
                    nkb = 4 * (qc + 1)
                    po0 = pqa.tile([65, 512], F32, name="po0", tag="po0", bufs=1)
                    po1 = pqa.tile([65, 512], F32, name="po1", tag="po1", bufs=1)
                    pos_ = (po0, po1)
                    for kb in range(nkb):
                        ck, tbk = 4 * b_ + kb // 4, kb % 4
                        # columns qq < 128*(kb-4qc) are fully masked for
                        # diagonal-strip blocks; skip them everywhere
                        qlo = max(0, 128 * (kb - 4 * qc))
                        qn = 512 - qlo
                        ps = pqa.tile([128, 2, 512], F32, name="ps", tag="ps",
                                      bufs=1)
                        for hh in range(HPC):
                            nc.tensor.matmul(
                                ps[:, hh, qlo:],
                                kT_sb[64 * hh:64 * hh + 64, ck,
                                      128 * tbk:128 * (tbk + 1)],
                                qT_sb[64 * hh:64 * hh + 64, gq, qlo:],
                                start=True, stop=True,
                                tile_position=(64 * hh, 0))
                        e_bf = att.tile([128, 2, 512], BF16, tag="ebf")
                        nc.scalar.activation(
                            out=e_bf[:, :, qlo:], in_=ps[:, :, qlo:],
                            func=AF.Exp, bias=neg4[:], scale=SCALE)
                        p_bf = att.tile([128, 2, 512], BF16, tag="pbf")
                        off = 2048 - (128 * kb - 512 * qc)
                        nc.vector.tensor_mul(
                            out=p_bf[:, :, qlo:], in0=e_bf[:, :, qlo:],
                            in1=expb_sb[:, :, off + qlo:off + 512])
                        for hh in range(HPC):
                            nc.tensor.matmul(
                                pos_[hh][:, qlo:],
                                v_sb[:, ck, tbk, 65 * hh:65 * (hh + 1)],
                                p_bf[:, hh, qlo:],
                                start=(kb == 0), stop=(kb == nkb - 1))
                    for hh in range(HPC):
                        o_sb = att.tile([65, 512], F32, tag="osb")
                        if hh:
                            nc.scalar.copy(out=o_sb[:], in_=pos_[hh][:])
                        else:
                            nc.vector.tensor_copy(out=o_sb[:], in_=pos_[hh][:])
                        for tb in range(4):
                            pt = pqa.tile([128, 65], F32, name="pt", tag="small",
                                          bufs=2)
                            nc.tensor.transpose(pt[:],
                                                o_sb[:, 128 * tb:128 * (tb + 1)],
                                                ident_f[:65, :65])
                            rc = att.tile([128, 1], F32, tag="rc")
                            nc.vector.reciprocal(out=rc[:], in_=pt[:, 64:65])
                            dst = attn_sb[:, gq, tb, 64 * hh:64 * (hh + 1)]
                            if tb % 2:
                                nc.scalar.activation(out=dst, in_=pt[:, 0:64],
                                                     func=AF.Copy, scale=rc[:])
                            else:
                                nc.vector.tensor_scalar_mul(out=dst,
                                                            in0=pt[:, 0:64],
                                                            scalar1=rc[:])
                    nc.sync.dma_start(
                        out=a2a_in[:].rearrange("(g tb p) c -> g p tb c",
                                                tb=4, p=128)[gq],
                        in_=attn_sb[:, gq, :, :])

                # snake order spreads the PE-dense kqv chunks across the
                # ACT/DVE-dense attention groups (deps: attn(gq) needs chunks
                # 4*(gq//4) .. gq)
                kqv_chunk(0)
                kqv_chunk(4)
                feed = [1, 5, 2, 6, 3, 7]
                for i, gq in enumerate([0, 4, 1, 5, 2, 6, 3, 7]):
                    attn_group(gq)
                    if i < len(feed):
                        kqv_chunk(feed[i])
            if debug:
                nc.sync.dma_start(out=dbg_attn[:], in_=attn_sb[:])
            nc.gpsimd.collective_compute(
                "AllToAll", mybir.AluOpType.bypass, replica_groups=RG,
                ins=[a2a_in.opt()], outs=[a2a_out.opt()])

            a2a_v = a2a_out[:].rearrange("(r tk p) c -> tk p r c", r=W, p=128)

            # =========== Phase 4: residual + LN2 + transpose ===========
            with (
                tc.tile_pool(name="afull", bufs=2) as afp,
                tc.tile_pool(name="pT2", bufs=4, space="PSUM") as pT2,
            ):
                for tt in range(4):
                    af = afp.tile([128, W, 128], BF16)
                    nc.sync.dma_start(out=af[:], in_=a2a_v[tt])
                    nc.vector.tensor_add(out=x_sb[:, tt, :], in0=x_sb[:, tt, :],
                                         in1=af[:].rearrange("p r c -> p (r c)"))
                    nc.vector.tensor_add(out=xb2_sb[:, tt, :], in0=x_sb[:, tt, :],
                                         in1=b2_b[:])
                    h2_bf = lnp.tile([128, E], BF16, tag="h2bf")
                    _ln(nc, lnp, x_sb[:, tt, :], g2_b, be2_b, h2_bf[:], eps_t)
                    for eb in range(8):
                        pt = pT2.tile([128, 128], BF16)
                        nc.tensor.transpose(pt[:], h2_bf[:, 128 * eb:128 * (eb + 1)],
                                            ident_bf[:])
                        if eb % 2:
                            nc.scalar.copy(out=h2T_sb[:, eb, 128 * tt:128 * (tt + 1)],
                                           in_=pt[:])
                        else:
                            nc.vector.tensor_copy(
                                out=h2T_sb[:, eb, 128 * tt:128 * (tt + 1)], in_=pt[:])

            # =========== Phase 5: fc1 (a^T = relu(w1^T h2^T + b1)) ===========
            with (
                tc.tile_pool(name="w1t", bufs=6) as w1p,
                tc.tile_pool(name="ps_a", bufs=1, space="PSUM") as ps_a,
            ):
                for dffc in range(8):
                    pa = [ps_a.tile([128, 512], F32, name=f"pa{i}", tag=f"pa{i}")
                          for i in range(4)]
                    for eb in range(8):
                        w1t = w1p.tile([128, 512], BF16)
                        nc.sync.dma_start(
                            out=w1t[:],
                            in_=w1b[128 * eb:128 * (eb + 1),
                                    512 * dffc:512 * (dffc + 1)])
                        for db in range(4):
                            nc.tensor.matmul(pa[db][:],
                                             w1t[:, 128 * db:128 * (db + 1)],
                                             h2T_sb[:, eb, :],
                                             start=(eb == 0), stop=(eb == 7))
                    for db in range(4):
                        j = 4 * dffc + db
                        nc.scalar.activation(out=aT_sb[:, j, :], in_=pa[db][:],
                                             func=AF.Relu, bias=b1_sb[:, j:j + 1],
                                             scale=1.0)

            # =========== Phase 6: fc2 + residual ===========
            with (
                tc.tile_pool(name="w2t", bufs=6) as w2p,
                tc.tile_pool(name="ps_f", bufs=1, space="PSUM") as ps_f,
                tc.tile_pool(name="outp", bufs=4) as outp,
            ):
                pf = [[ps_f.tile([128, 512], F32, name=f"pf{i}_{j}", tag=f"pf{i}_{j}")
                       for j in range(2)] for i in range(4)]
                for db in range(32):
                    w2t = w2p.tile([128, E], BF16)
                    nc.sync.dma_start(out=w2t[:], in_=w2b[128 * db:128 * (db + 1), :])
                    for tt in range(4):
                        for eh in range(2):
                            nc.tensor.matmul(pf[tt][eh][:],
                                             aT_sb[:, db, 128 * tt:128 * (tt + 1)],
                                             w2t[:, 512 * eh:512 * (eh + 1)],
                                             start=(db == 0), stop=(db == 31))
                for tt in range(4):
                    for eh in range(2):
                        ot = outp.tile([128, 512], F32)
                        nc.vector.tensor_add(
                            out=ot[:], in0=pf[tt][eh][:],
                            in1=xb2_sb[:, tt, 512 * eh:512 * (eh + 1)])
                        nc.sync.dma_start(
                            out=out_ext[128 * tt:128 * (tt + 1),
                                        512 * eh:512 * (eh + 1)],
                            in_=ot[:])
    nc.compile()
    return nc


def _expbias_table():
    """[HPC, 128, 4096] per-core list: exp(slope*(k - c + 2048)), 0 where future."""
    kk = np.arange(128)[:, None]
    cc = np.arange(4096)[None, :]
    rel = (kk - cc + 2048).astype(np.float64)
    xslope = (2.0 ** 8) ** (1.0 / H)
    tables = []
    for r in range(W):
        heads = []
        for hh in range(HPC):
            h = 2 * r + hh
            slope = 1.0 / xslope ** (h + 1)
            with np.errstate(over="ignore"):
                g = np.where(rel <= 0, np.exp(slope * rel), 0.0)
            heads.append(g.astype(np.float32))
        tables.append(np.stack(heads).astype(ml_dtypes.bfloat16))
    return tables


def kernel(x, w_kqv, ln1_g, ln1_b, ln2_g, ln2_b, w1, b1, w2, b2, _debug=False):
    x = np.asarray(x, np.float32)
    x_flat = np.ascontiguousarray(x.reshape(T, E))
    wk = np.asarray(w_kqv, np.float32)
    bf = ml_dtypes.bfloat16
    w1_bf = np.ascontiguousarray(np.asarray(w1, np.float32).astype(bf))
    w2_bf = np.ascontiguousarray(np.asarray(w2, np.float32).astype(bf))

    key = "nc_dbg" if _debug else "nc"
    if key not in _CACHE:
        _CACHE[key] = _build(debug=_debug)
        _CACHE.setdefault("expb", _expbias_table())
    nc = _CACHE[key]
    expbs = _CACHE["expb"]

    in_maps = []
    for r in range(W):
        cols = slice(128 * r, 128 * (r + 1))
        wqk_r = np.concatenate([wk[:, 0:E][:, cols], wk[:, E:2 * E][:, cols]],
                               axis=1).astype(bf)
        wv_r = np.ascontiguousarray(wk[:, 2 * E:3 * E][:, cols]).astype(bf)
        in_maps.append({
            "x_s": np.ascontiguousarray(x_flat[TPC * r:TPC * (r + 1)]),
            "wqk": np.ascontiguousarray(wqk_r),
            "wv": wv_r,
            "w1b": w1_bf,
            "w2b": w2_bf,
            "b1": np.ascontiguousarray(np.asarray(b1, np.float32)),
            "b2": np.ascontiguousarray(np.asarray(b2, np.float32)),
            "g1": np.ascontiguousarray(np.asarray(ln1_g, np.float32)),
            "be1": np.ascontiguousarray(np.asarray(ln1_b, np.float32)),
            "g2": np.ascontiguousarray(np.asarray(ln2_g, np.float32)),
            "be2": np.ascontiguousarray(np.asarray(ln2_b, np.float32)),
            "expb": expbs[r],
        })

    _CACHE["last_in_maps"] = in_maps
    res = run_bass_kernel_spmd(nc, in_maps, core_ids=list(range(W)))
    out = np.concatenate([res.results[r]["out"] for r in range(W)], axis=0)
    if _debug:
        return out.reshape(B, S, E), res.results
    return out.reshape(B, S, E)


if __name__ == "__main__":
    import reference
    inputs = {k: np.asarray(v) for k, v in reference.setup_inputs().items()}
    got = kernel(**inputs)
    exp = np.asarray(reference.reference(**reference.setup_inputs()))
    err = np.abs(got - exp).max() / np.abs(exp).max()
    print("Relative error:", err)
